# revision 1
# baseline (speedup 1.0000x reference)
"""Trainium2 Bass kernel for nn_Decoder_58514634440787 (histogram_binning).

Piecewise-linear decoder: y[b, s] = interp of (segment_x, segment_y) knots
evaluated at the uniform pixel grid t_s = (s+1)/S, S = 196608, B = 8.

The output along the pixel axis is piecewise linear with at most 33 knots
per batch.  Pixels are sharded across 8 cores (24576 each) and laid out
on-chip as [128 partitions = 8 batches x 16 rows, 1536 pixels].  Each
1536-pixel row intersects at most a couple of knots.  The host converts the
tiny [8, 33] knot tensors into per-row line parameters in *column space*
(f = 0..1535 within the row; t = (s0 + f + 1)/S is folded into slope and
intercept in float64), so the device only needs:

    f    = iota along the free axis                  (GPSIMD, exact in f32)
    out  = aB[p]*f + bB[p]                           (base line)
    patch left  half where f <  colL[p] with line (aL, bL)
    patch right half where f >= colR[p] with line (aR, bR)

Masks are integer compares against host-computed breakpoint columns (exact:
the host does searchsorted on the exact f32 grid, the same predicate the
reference evaluates), and line evaluation runs on the Scalar/Vector engines
as per-partition scale+bias (the Scalar engine fuses f*a+b in one rounding).
No big input tensor: the only DMA in is a small transposed parameter block
[rows, 128+rows] (32 wide descriptors instead of 128 tiny ones, identity
appended) which the otherwise-idle TensorEngine un-transposes via an
identity matmul.  Output stores are split across both HWDGE queues
(sync + scalar) to double store bandwidth.

The number of patch slots per half-row adapts to the data ((1,1) for the
benchmark inputs); each additional slot adds one mask + line + predicated
copy.  Correct for any input with at most ~20 breakpoint slots per
768-pixel half-row (beyond that _host_prep asserts).

Inputs are the full [8, 33] knot tensors; sharding/gather happens here.
Measured on 8 axon trn2 cores: HW exec time ~18.3 us, relative error
(norm) 1.7e-7, max elementwise 2.2e-4 vs the jax-cpu reference.
"""

import numpy as np

S = 196608
B = 8
W = 1536              # pixels per partition row
RPB = 16              # rows per batch per core
P = 128               # partitions = B * RPB
NCORES = 8
PIX_PER_CORE = RPB * W  # 24576
HALF = W // 2

_t_grid = None          # f32 [S] exact (s+1)/S
_compiled = {}          # (n_left, n_right) -> nc


def _get_grid():
    global _t_grid
    if _t_grid is None:
        _t_grid = (np.arange(1, S + 1, dtype=np.float64) / S).astype(np.float32)
    return _t_grid


def _fix_x_order(sx, sy):
    """Running max of x along the segment axis, y carried from the position
    achieving the max (ties keep the later entry). Matches reference."""
    x = sx.copy()
    y = sy.copy()
    for b in range(sx.shape[0]):
        cx, cy = sx[b, 0], sy[b, 0]
        for i in range(sx.shape[1]):
            if sx[b, i] >= cx:
                cx, cy = sx[b, i], sy[b, i]
            x[b, i] = cx
            y[b, i] = cy
    return x, y


def _host_prep(segment_x, segment_y):
    """Returns (pT_per_core, (n_left, n_right)).

    pT_per_core: [32, 128] f32; row j holds parameter j for all 128
    partitions.  Parameters per partition row (iota-column space):
      [aB, bB, (colL_j, aL_j, bL_j)..., (colR_j, aR_j, bR_j)...]
    Left slots are ordered latest-breakpoint-first; right slots
    earliest-first.  line(f) = a*f + b  with  a = ratio/S  and
    b = ratio*((s0+1)/S - x_m) + y_m  computed in float64.
    """
    t_grid = _get_grid()
    sx = np.asarray(segment_x, dtype=np.float32)
    sy = np.asarray(segment_y, dtype=np.float32)
    x, y = _fix_x_order(sx, sy)

    gaps = x[:, 1:] - x[:, :-1]
    div = np.where(gaps == 0.0, np.float32(0.0001), gaps).astype(np.float32)
    a = ((y[:, 1:] - y[:, :-1]) / div).astype(np.float32)          # [B, 32]
    a64 = a.astype(np.float64)
    x64 = x.astype(np.float64)
    y64 = y.astype(np.float64)

    # First pixel index s with t_s >= x_n, for binning knots n = 1..31.
    # searchsorted on the exact f32 grid == the reference's f32 compares.
    k = np.stack([np.searchsorted(t_grid, x[b, 1:32], side='left')
                  for b in range(B)])                               # [B, 31]

    # per (batch, global row): breakpoints, dedup by pixel keeping largest n
    rows = [[dict() for _ in range(NCORES * RPB)] for _ in range(B)]
    for b in range(B):
        for n in range(31):
            kk = int(k[b, n])
            if kk < S:
                rows[b][kk // W][kk % W] = n + 1   # knot index 1..31
    ks = [np.sort(k[b]) for b in range(B)]

    def seg(b, s):
        # segment index at pixel s = number of breakpoints with k <= s
        return int(np.searchsorted(ks[b], s, side='right'))

    def line(b, m, s0):
        # (slope, intercept) in local column space for segment m of batch b,
        # for a span starting at global pixel s0 (f local to that span)
        aa = a64[b, m]
        bb = aa * ((s0 + 1) / S - x64[b, m]) + y64[b, m]
        return (np.float32(aa / S), np.float32(bb))

    n_left = n_right = 0
    per_row = []
    for c in range(NCORES):
        core_rows = []
        for b in range(B):
            for r in range(RPB):
                g = c * RPB + r
                s0 = c * PIX_PER_CORE + r * W
                bps = sorted(rows[b][g].items())   # [(col, knot_n)...]
                left = [(col, n) for col, n in bps if col < HALF]
                right = [(col, n) for col, n in bps if col >= HALF]
                n_left = max(n_left, len(left))
                n_right = max(n_right, len(right))
                mb = seg(b, s0 + HALF - 1)
                baseL = line(b, mb, s0)             # f in [0, HALF)
                baseR = line(b, mb, s0 + HALF)[1]   # same slope, right span
                lslots = []
                for col, n in sorted(left, reverse=True):
                    m_prev = seg(b, s0 + col - 1)
                    lslots.append((np.float32(col),) + line(b, m_prev, s0))
                rslots = []
                for col, n in sorted(right):
                    m_at = seg(b, s0 + col)
                    rslots.append((np.float32(col - HALF),)
                                  + line(b, m_at, s0 + HALF))
                core_rows.append((baseL, baseR, lslots, rslots))
        per_row.append(core_rows)

    n_left = max(n_left, 1)
    n_right = max(n_right, 1)
    ncol = 3 + 3 * (n_left + n_right)
    rows = _prm_rows(n_left, n_right)
    pTs = []
    for c in range(NCORES):
        arr = np.zeros((rows, P + rows), dtype=np.float32)
        for p, (baseL, baseR, lslots, rslots) in enumerate(per_row[c]):
            vals = [baseL[0], baseL[1], baseR]
            for j in range(n_left):
                vals += list(lslots[j]) if j < len(lslots) else [-1.0, 0.0, 0.0]
            for j in range(n_right):
                vals += list(rslots[j]) if j < len(rslots) else [4096.0, 0.0, 0.0]
            arr[:len(vals), p] = vals
        arr[:, P:] = np.eye(rows, dtype=np.float32)  # identity for PE transpose
        pTs.append(arr)
    return pTs, (n_left, n_right)


def _prm_rows(n_left, n_right):
    """Partition rows of the transposed parameter block (even, = used cols)."""
    ncol = 3 + 3 * (n_left + n_right)
    rows = ncol + (ncol & 1)
    assert rows <= 128, f"too many breakpoint slots for one row: {ncol}"
    return rows


def _build(n_left, n_right):
    import concourse.bacc as bacc
    import concourse.mybir as mybir
    from concourse.tile import TileContext

    f32 = mybir.dt.float32
    Alu = mybir.AluOpType
    Act = mybir.ActivationFunctionType
    rows = _prm_rows(n_left, n_right)

    nc = bacc.Bacc("TRN2", debug=False, enable_asserts=False,
                   enable_partition_id=False, monotonic_sem_count=0)
    pT_dram = nc.dram_tensor("pT", [rows, P + rows], f32,
                             kind="ExternalInput").ap()
    y_dram = nc.dram_tensor("y", [P, W], f32, kind="ExternalOutput").ap()

    with TileContext(nc) as tc:
        with tc.tile_pool(name="pool", bufs=1) as pool, \
             tc.tile_pool(name="psum", bufs=1, space="PSUM") as psum_pool:
            # params arrive transposed ([32, 128]: 32 big descriptors instead
            # of 128 tiny ones); the idle PE un-transposes them via an
            # identity matmul.
            pT = pool.tile([rows, P + rows], f32, name="pT_t", tag="pT_t")
            nc.sync.dma_start(out=pT[:], in_=pT_dram[:])
            prm_ps = psum_pool.tile([P, rows], f32, name="prm_ps", tag="prm_ps")
            nc.tensor.transpose(prm_ps[:], pT[:, :P], pT[:, P:])
            prm = pool.tile([P, rows], f32, name="prm", tag="prm")
            nc.vector.tensor_copy(out=prm[:], in_=prm_ps[:])

            def sc(j):  # scalar AP = params column j
                return prm[:, j:j + 1]

            # warm the activation table off the critical path
            warm = pool.tile([P, 2], f32, name="warm", tag="warm")
            nc.vector.memset(warm[:], 0.0)
            nc.scalar.activation(warm[:, 1:2], warm[:, 0:1], Act.Identity)

            # local column index 0..HALF-1, shared by both halves (the host
            # expresses the right half in its own local coordinates)
            io = pool.tile([P, HALF], f32, name="io", tag="io")
            nc.gpsimd.iota(io[:], pattern=[[1, HALF]], base=0,
                           channel_multiplier=0,
                           allow_small_or_imprecise_dtypes=True)
            t = io[:]

            for h in range(2):
                o = pool.tile([P, HALF], f32, name=f"o{h}", tag=f"o{h}")
                # base line: o = f*aB + bB  (Scalar engine, fused FMA)
                nc.scalar.activation(o[:], t, Act.Identity,
                                     bias=sc(1 + h), scale=sc(0))
                if h == 0:
                    slots = [(3 + 3 * j, Alu.is_lt) for j in range(n_left)]
                else:
                    slots = [(3 + 3 * (n_left + j), Alu.is_ge)
                             for j in range(n_right)]
                for si, (bc, cmp_op) in enumerate(slots):
                    m = pool.tile([P, HALF], mybir.dt.uint8,
                                  name=f"m{h}{si}", tag=f"m{h}{si}")
                    ln = pool.tile([P, HALF], f32,
                                   name=f"l{h}{si}", tag=f"l{h}{si}")
                    # masks on DVE, lines split ACT/DVE
                    nc.vector.tensor_scalar(m[:], t, sc(bc), None, cmp_op)
                    if h == 0:
                        nc.vector.tensor_scalar(ln[:], t, sc(bc + 1),
                                                sc(bc + 2), Alu.mult, Alu.add)
                    else:
                        nc.scalar.activation(ln[:], t, Act.Identity,
                                             bias=sc(bc + 2), scale=sc(bc + 1))
                    # split the merges so stores can start sooner
                    nc.vector.copy_predicated(o[:, :576], m[:, :576],
                                              ln[:, :576])
                    nc.vector.copy_predicated(o[:, 576:], m[:, 576:],
                                              ln[:, 576:])
                # stores: balance the two HWDGE queues (384 KB each)
                c0 = h * HALF
                if h == 0:
                    nc.sync.dma_start(out=y_dram[:, c0:c0 + 576],
                                      in_=o[:, :576])
                    nc.scalar.dma_start(out=y_dram[:, c0 + 576:c0 + HALF],
                                        in_=o[:, 576:])
                else:
                    nc.scalar.dma_start(out=y_dram[:, c0:c0 + 576],
                                        in_=o[:, :576])
                    nc.sync.dma_start(out=y_dram[:, c0 + 576:c0 + HALF],
                                      in_=o[:, 576:])

    nc.compile()
    return nc


def _get_compiled(n_left, n_right):
    key = (n_left, n_right)
    if key not in _compiled:
        _compiled[key] = _build(n_left, n_right)
    return _compiled[key]


def kernel(segment_x, segment_y):
    from concourse.bass_utils import run_bass_kernel_spmd

    pTs, (n_left, n_right) = _host_prep(segment_x, segment_y)
    nc = _get_compiled(n_left, n_right)
    in_maps = [{"pT": pTs[c]} for c in range(NCORES)]
    res = run_bass_kernel_spmd(nc, in_maps, core_ids=list(range(NCORES)))

    out = np.empty((B, S), dtype=np.float32)
    for c in range(NCORES):
        yc = res.results[c]["y"]  # [128, 1536]
        base = c * PIX_PER_CORE
        out[:, base:base + PIX_PER_CORE] = yc.reshape(B, RPB * W)
    return out



# revision 7
# speedup vs baseline: 1.0087x; 1.0087x over previous
"""Trainium2 Bass kernel for nn_Decoder_58514634440787 (histogram_binning).

Piecewise-linear decoder: y[b, s] = interp of (segment_x, segment_y) knots
evaluated at the uniform pixel grid t_s = (s+1)/S, S = 196608, B = 8.

Pixels are sharded across 8 cores (24576 each), laid out on-chip as
[128 partitions = 8 batches x 16 rows, 1536 pixels].  The pixel axis is
processed in column chunks (512/512/384/128).  For each chunk the host
builds a tiny piecewise-linear *basis* so the whole computation is one
single-pass fp16 matmul per chunk on the otherwise-idle TensorEngine:

    out[p, f] = a[p]*f + b[p] + sum_j w_j[p] * D_j[f]
    out_psum[128, C] = lhsT[R, 128].T @ M[R, C]

(a, b) is the line of the segment active at the chunk start for
partition p (rebased to chunk-local f, float64 host math), and each
basis row D_j covers one segment transition at chunk-local column k_j:
D_j[f] = (f >= k_j) ? (dalpha*f + dbeta) : 0 is the difference between
new and old segment lines, with w_j one-hot on the owning partition.
Summed left to right the deltas telescope, so every integer f gets
exactly the active segment's line -- jumps from zero-width segments
included, no continuity assumption.

fp16 keeps the matmul single-pass (fp32 needs two LOW/HIGH passes and
was 4x slower end-to-end on the PE).  Accuracy is preserved by:
  - the iota row is f * 2^-e with integer f -- exact in fp16;
  - every value row is split into hi/lo fp16 rows (hi = fp16(v),
    lo = fp16(v - hi)), recovering ~22-bit precision;
  - per-row power-of-2 scaling (folded into the paired lhsT entry,
    powers of two are exact) keeps magnitudes out of the subnormal
    range where fp16 rounding error would blow up.

Each chunk's [R, 128+C] fp16 block arrives as one small DMA (~10-20 KB)
split over the two HWDGE queues; PSUM->SBUF copies alternate between
the Vector and Scalar engines; stores alternate between the GpSimd
(SWDGE) and Sync (HWDGE) queues so no engine issues more than two DMA
triggers.  R adapts to the data and is bucketed for compile caching; a
chunk whose transition count would overflow the 128-partition
contraction limit is split column-wise, so any input fits.

Inputs are the full [8, 33] knot tensors; sharding/gather happens here.
"""

import numpy as np

S = 196608
B = 8
W = 1536              # pixels per partition row
RPB = 16              # rows per batch per core
P = 128               # partitions = B * RPB
NCORES = 8
PIX_PER_CORE = RPB * W  # 24576

_t_grid = None          # f32 [S] exact (s+1)/S
_compiled = {}          # layout tuple -> nc

_R_BUCKETS = (8, 12, 16, 24, 32, 48, 64, 96, 128)
_BASE_WIDTHS = (256, 512, 512, 256)


def _get_grid():
    global _t_grid
    if _t_grid is None:
        _t_grid = (np.arange(1, S + 1, dtype=np.float64) / S).astype(np.float32)
    return _t_grid


def _fix_x_order(sx, sy):
    """Running max of x along the segment axis, y carried from the position
    achieving the max (ties keep the later entry). Matches reference."""
    x = sx.copy()
    y = sy.copy()
    for b in range(sx.shape[0]):
        cx, cy = sx[b, 0], sy[b, 0]
        for i in range(sx.shape[1]):
            if sx[b, i] >= cx:
                cx, cy = sx[b, i], sy[b, i]
            x[b, i] = cx
            y[b, i] = cy
    return x, y


def _pow2_scale(vmax):
    """Power-of-two s with vmax*s ~ 2^4, so fp16(v*s) avoids subnormals
    and overflow.  The inverse 2^-e (|e| <= 24) is exact in fp16, and so
    is f * 2^-e for integer f <= 2047 (subnormals included)."""
    if vmax <= 0.0 or not np.isfinite(vmax):
        return 1.0
    e = 4 - int(np.floor(np.log2(vmax)))
    e = min(max(e, -10), 24)
    return float(2.0 ** e)


def _host_prep(segment_x, segment_y):
    """Returns (in_maps_arrays, layout).

    layout: tuple of (chunk_width, R) pairs, same for every core.
    in_maps_arrays: [core][chunk] -> fp16 [R, 128 + chunk_width] array
    holding lhsT (cols 0:128) and the moving basis M (cols 128:).
    """
    t_grid = _get_grid()
    sx = np.asarray(segment_x, dtype=np.float32)
    sy = np.asarray(segment_y, dtype=np.float32)
    x, y = _fix_x_order(sx, sy)

    gaps = x[:, 1:] - x[:, :-1]
    div = np.where(gaps == 0.0, np.float32(0.0001), gaps).astype(np.float32)
    a = ((y[:, 1:] - y[:, :-1]) / div).astype(np.float32)          # [B, 32]
    a64 = a.astype(np.float64)
    x64 = x.astype(np.float64)
    y64 = y.astype(np.float64)

    # First pixel index s with t_s >= x_n, for knots n = 1..31.
    # searchsorted on the exact f32 grid == the reference's f32 compares.
    k = np.stack([np.searchsorted(t_grid, x[b, 1:32], side='left')
                  for b in range(B)])                               # [B, 31]
    ks = [np.sort(k[b]) for b in range(B)]

    def seg(b, s):
        # segment index at pixel s = number of transition pixels <= s
        return int(np.searchsorted(ks[b], s, side='right'))

    def line64(b, m, s0):
        # (slope, intercept) in local column space for segment m of batch b,
        # f local to a span starting at global pixel s0, in float64
        aa = a64[b, m]
        return (aa / S, aa * ((s0 + 1) / S - x64[b, m]) + y64[b, m])

    # transitions[b] = sorted unique global pixels where the segment changes
    transitions = [np.unique(k[b][k[b] < S]) for b in range(B)]

    def chunk_transitions(widths):
        """[core][chunk] -> list of (p, k_loc, b, s0) transition entries.
        k_loc in [1, width): a transition at the chunk start is folded
        into the base line."""
        offs = np.concatenate([[0], np.cumsum(widths)]).astype(int)
        out = [[[] for _ in widths] for _ in range(NCORES)]
        for c in range(NCORES):
            for b in range(B):
                for r in range(RPB):
                    row0 = c * PIX_PER_CORE + r * W
                    p = b * RPB + r
                    tr = transitions[b]
                    lo = np.searchsorted(tr, row0, side='right')
                    hi = np.searchsorted(tr, row0 + W - 1, side='right')
                    for g in tr[lo:hi]:
                        col = int(g) - row0          # 1..W-1
                        ci = int(np.searchsorted(offs, col, side='right')) - 1
                        k_loc = col - int(offs[ci])
                        if k_loc == 0:
                            continue  # covered by that chunk's base line
                        out[c][ci].append((p, k_loc, b, row0 + int(offs[ci])))
        return out

    # choose chunk widths so every (core, chunk) fits 2*(2+n)<=128 rows
    widths = list(_BASE_WIDTHS)
    while True:
        per = chunk_transitions(widths)
        worst = [max(len(per[c][i]) for c in range(NCORES))
                 for i in range(len(widths))]
        bad = [i for i, n in enumerate(worst) if 2 * (2 + n) > 128]
        if not bad:
            break
        i = bad[0]
        w = widths[i]
        assert w >= 2, "cannot split further"
        widths = widths[:i] + [w // 2, w - w // 2] + widths[i + 1:]

    rs = []
    for i, n in enumerate(worst):
        need = 2 * (2 + n)
        rb = next(rr for rr in _R_BUCKETS if rr >= need)
        rs.append(rb)
    layout = tuple(zip(widths, rs))

    def hilo(v64):
        """Split float64 array/scalar into (hi, lo) fp16 pair."""
        hi = np.asarray(v64, dtype=np.float16)
        lo = np.asarray(v64 - hi.astype(np.float64), dtype=np.float16)
        return hi, lo

    offs = np.concatenate([[0], np.cumsum(widths)]).astype(int)
    arrays = []
    for c in range(NCORES):
        core_arrays = []
        for i, (cw, R) in enumerate(layout):
            arr = np.zeros((R, 128 + cw), dtype=np.float16)
            f = np.arange(cw, dtype=np.float64)

            # base lines per partition, rebased to this chunk's start
            av = np.zeros(P, dtype=np.float64)
            bv = np.zeros(P, dtype=np.float64)
            for b in range(B):
                for r in range(RPB):
                    p = b * RPB + r
                    s0 = c * PIX_PER_CORE + r * W + int(offs[i])
                    al, be = line64(b, seg(b, s0), s0)
                    av[p] = al
                    bv[p] = be

            # rows 0-1: a_hi/a_lo (in lhsT) paired with scaled iota (in M)
            sa = _pow2_scale(float(np.max(np.abs(av))))
            iota = (f / sa).astype(np.float16)  # f * 2^-e: exact
            ahi, alo = hilo(av * sa)
            arr[0, :128] = ahi
            arr[1, :128] = alo
            arr[0, 128:] = iota
            arr[1, 128:] = iota

            # rows 2-3: b_hi/b_lo paired with a scaled constant row
            sb = _pow2_scale(float(np.max(np.abs(bv))))
            bhi, blo = hilo(bv * sb)
            arr[2, :128] = bhi
            arr[3, :128] = blo
            arr[2, 128:] = np.float16(1.0 / sb)
            arr[3, 128:] = np.float16(1.0 / sb)

            # two rows per transition: D_hi / D_lo, one-hot scaled lhsT
            for j, (p, k_loc, b, s0) in enumerate(per[c][i]):
                m_new = seg(b, s0 + k_loc)
                m_old = seg(b, s0 + k_loc - 1)
                al_n, be_n = line64(b, m_new, s0)
                al_o, be_o = line64(b, m_old, s0)
                d = np.where(f >= k_loc,
                             (al_n - al_o) * f + (be_n - be_o), 0.0)
                sd = _pow2_scale(float(np.max(np.abs(d))))
                dhi, dlo = hilo(d * sd)
                arr[4 + 2 * j, 128:] = dhi
                arr[5 + 2 * j, 128:] = dlo
                arr[4 + 2 * j, p] = np.float16(1.0 / sd)
                arr[5 + 2 * j, p] = np.float16(1.0 / sd)
            core_arrays.append(arr)
        arrays.append(core_arrays)
    return arrays, layout


def _build(layout):
    import concourse.bacc as bacc
    import concourse.mybir as mybir
    from concourse.tile import TileContext

    f16 = mybir.dt.float16
    f32 = mybir.dt.float32

    nc = bacc.Bacc("TRN2", debug=False, enable_asserts=False,
                   enable_partition_id=False, monotonic_sem_count=0)
    ins = [nc.dram_tensor(f"pm{i}", [R, 128 + cw], f16,
                          kind="ExternalInput").ap()
           for i, (cw, R) in enumerate(layout)]
    y_dram = nc.dram_tensor("y", [P, W], f32, kind="ExternalOutput").ap()

    with TileContext(nc) as tc:
        with tc.tile_pool(name="pool", bufs=1) as pool, \
             tc.tile_pool(name="psum", bufs=1, space="PSUM") as psum_pool:
            tiles = []
            for i, (cw, R) in enumerate(layout):
                tin = pool.tile([R, 128 + cw], f16, name=f"pm{i}",
                                tag=f"pm{i}")
                eng = nc.sync if i % 2 == 0 else nc.scalar
                eng.dma_start(out=tin[:], in_=ins[i][:])
                tiles.append(tin)
            # store queues: first store on HWDGE (prompt drain start),
            # middle stores on SWDGE (its ~1.2us doorbell latency hides
            # behind the already-draining first store), last on HWDGE so
            # the final completion isn't delayed by the SWDGE latency.
            store_eng = [nc.sync, nc.gpsimd, nc.gpsimd, nc.scalar]
            off = 0
            for i, (cw, R) in enumerate(layout):
                ps = psum_pool.tile([P, cw], f32, name=f"ps{i}", tag=f"ps{i}")
                nc.tensor.matmul(ps[:], tiles[i][:, :128], tiles[i][:, 128:])
                ot = pool.tile([P, cw], f32, name=f"o{i}", tag=f"o{i}")
                if i % 2 == 0:
                    nc.vector.tensor_copy(out=ot[:], in_=ps[:])
                else:
                    nc.scalar.copy(out=ot[:], in_=ps[:])
                store_eng[i % 4].dma_start(out=y_dram[:, off:off + cw],
                                           in_=ot[:])
                off += cw

    nc.compile()
    return nc


def _get_compiled(layout):
    if layout not in _compiled:
        _compiled[layout] = _build(layout)
    return _compiled[layout]


def kernel(segment_x, segment_y):
    from concourse.bass_utils import run_bass_kernel_spmd

    arrays, layout = _host_prep(segment_x, segment_y)
    nc = _get_compiled(layout)
    in_maps = [{f"pm{i}": arrays[c][i] for i in range(len(layout))}
               for c in range(NCORES)]
    res = run_bass_kernel_spmd(nc, in_maps, core_ids=list(range(NCORES)))

    out = np.empty((B, S), dtype=np.float32)
    for c in range(NCORES):
        yc = res.results[c]["y"]  # [128, 1536]
        base = c * PIX_PER_CORE
        out[:, base:base + PIX_PER_CORE] = yc.reshape(B, RPB * W)
    return out
